# revision 6
# baseline (speedup 1.0000x reference)
"""Distributed Trainium2 kernel for 4-relation GNN message passing.

out = sum_r segment_sum(x[src_r] @ W_r.T + b_r, tgt_r) + x @ W_sl.T + b_sl

Strategy (8 NeuronCores, nodes sharded 12500/core):
  - Algebra: hoist the per-edge linear out of the segment sum:
      segment_sum(x[src] @ W.T + b) = segment_sum(x[src]) @ W.T + deg * b
    so the device only gathers raw features, scatter-adds them into a
    per-(relation, node) accumulator A_r, and applies a small matmul
    epilogue  sum_r A_r @ W_r.T + deg_r b_r + x W_sl.T + b_sl.
  - x is replicated to every core at input-staging time (not timed), so no
    collectives are needed; each core owns the edges whose target falls in
    its node shard.
  - Gather: gpsimd.dma_gather (int16 indices -> src split into chunks of
    32000 rows).  Scatter: gpsimd.dma_scatter_add into an HBM accumulator
    (CCE fp32 add).  Tokens are lane-packed host-side with lane = tgt % 16
    so all duplicates of a target serialize on one DMA engine (the ucode
    assigns token slot j -> engine 2*((j%32)>>2)+(j>=64), 8 slots/engine
    per 128-token block) -- otherwise concurrent read-modify-write of the
    same accumulator row races.
  - Epilogue: load A_r tiles, PE-transpose via identity matmul, accumulate
    W_r.T contributions in PSUM along with a K=5 matmul of stacked biases
    against [deg_0..deg_3, ones] and the self-loop term, then DMA out.
"""

import numpy as np

import concourse.bass as bass
import concourse.mybir as mybir
from concourse import bacc, tile
from concourse import library_config

F32 = mybir.dt.float32
I16 = mybir.dt.int16

N_NODES = 100000
D = 128
N_REL = 4
N_CORES = 8
NPC = N_NODES // N_CORES  # nodes per core
SRC_CHUNK = 32000  # int16 index limit per gather instruction
N_CHUNKS = (N_NODES + SRC_CHUNK - 1) // SRC_CHUNK
TRASH = NPC  # accumulator trash row for dummy-padding tokens
T_MAX = 1024  # tokens per gather/scatter instruction (2048+ overflows the SWDGE desc ring)
DEBUG_A = False
NODE_CHUNK = 512  # epilogue psum width

def _pack_core(edge_indices, m):
    """Per-core host prep: per-(relation, src_chunk) bins split into waves.

    Wave k of a bin holds the k-th occurrence of each target, so every
    scatter instruction has UNIQUE targets.  The CCE read-modify-write
    pipeline does not interlock same-address descriptors within one
    instruction (measured), but Tile's WAW ordering between scatter
    instructions waits for full DMA completion, which is safe (measured).
    Returns: bins: dict[(r, c)] -> list of (src_local, tgt_local) waves
             deg: [N_REL+1, NPC] f32  (deg per relation + ones row)
    """
    lo, hi = m * NPC, (m + 1) * NPC
    deg = np.zeros((N_REL + 1, NPC), np.float32)
    deg[N_REL] = 1.0
    bins = {}
    for r in range(N_REL):
        src = edge_indices[r, 0]
        tgt = edge_indices[r, 1]
        mask = (tgt >= lo) & (tgt < hi)
        s = src[mask].astype(np.int64)
        t = (tgt[mask] - lo).astype(np.int64)
        deg[r] = np.bincount(t, minlength=NPC).astype(np.float32)
        c = s // SRC_CHUNK
        for ci in range(N_CHUNKS):
            sel = c == ci
            sl = s[sel] - ci * SRC_CHUNK
            tl = t[sel]
            order = np.argsort(tl, kind="stable")
            sl, tl = sl[order], tl[order]
            n = len(tl)
            waves = []
            if n:
                firsts = np.r_[True, tl[1:] != tl[:-1]]
                idx_first = np.maximum.accumulate(
                    np.where(firsts, np.arange(n), 0)
                )
                rank = np.arange(n) - idx_first
                for k in range(int(rank.max()) + 1):
                    wsel = rank == k
                    ws, wt = sl[wsel], tl[wsel]
                    o2 = np.argsort(ws, kind="stable")  # src order: HBM locality
                    waves.append((ws[o2], wt[o2]))
            bins[(r, ci)] = waves
    return bins, deg


def _layout_wave(ws, wt, ntok):
    """Pad a wave to ntok tokens (int16 arrays)."""
    src_slots = np.zeros(ntok, np.int16)
    tgt_slots = np.full(ntok, TRASH, np.int16)
    src_slots[: len(ws)] = ws.astype(np.int16)
    tgt_slots[: len(wt)] = wt.astype(np.int16)
    return src_slots, tgt_slots


def _wrap16(a):
    """Token i -> [i % 16, i // 16], replicated to 128 partitions."""
    w = a.reshape(-1, 16).T
    return np.ascontiguousarray(np.tile(w, (8, 1)))


def prepare_inputs(x, edge_indices, W, b, W_sl, b_sl):
    """Host-side sharding/packing. Returns (in_maps, schedule)."""
    x = np.ascontiguousarray(x, dtype=np.float32)
    per_core = [_pack_core(edge_indices, m) for m in range(N_CORES)]

    # Equalize wave counts/sizes across cores (SPMD: one graph for all).
    schedule = []  # (r, c, tok_offset, num_idxs) shared by all cores
    wave_caps = {}  # (r, c) -> list of token caps (128-multiples)
    for r in range(N_REL):
        for c in range(N_CHUNKS):
            n_waves = max(len(pc[0][(r, c)]) for pc in per_core)
            caps = []
            for k in range(n_waves):
                mx = max(
                    len(pc[0][(r, c)][k][0]) if k < len(pc[0][(r, c)]) else 0
                    for pc in per_core
                )
                caps.append(-(-mx // 128) * 128)
            wave_caps[(r, c)] = caps
    tok_off = 0
    for r in range(N_REL):
        for c in range(N_CHUNKS):
            for cap in wave_caps[(r, c)]:
                o = 0
                while o < cap:
                    step = min(T_MAX, cap - o)
                    schedule.append((r, c, tok_off + o, step))
                    o += step
                tok_off += cap
    total_tokens = tok_off

    wts = np.ascontiguousarray(
        np.stack([W[r].T for r in range(N_REL)] + [W_sl.T]), dtype=np.float32
    )  # [5, 128, 128] lhsT layout [fin, fout]
    bstack = np.ascontiguousarray(
        np.concatenate([b, b_sl[None, :]], axis=0), dtype=np.float32
    )  # [5, 128]
    ident = np.eye(128, dtype=np.float32)

    in_maps = []
    for m in range(N_CORES):
        bins, deg = per_core[m]
        src_all = np.zeros(total_tokens, np.int16)
        tgt_all = np.full(total_tokens, TRASH, np.int16)
        off = 0
        for r in range(N_REL):
            for c in range(N_CHUNKS):
                waves = bins[(r, c)]
                for k, cap in enumerate(wave_caps[(r, c)]):
                    if k < len(waves):
                        ss, ts = _layout_wave(waves[k][0], waves[k][1], cap)
                        src_all[off : off + cap] = ss
                        tgt_all[off : off + cap] = ts
                    off += cap
        xt = np.ascontiguousarray(x[m * NPC : (m + 1) * NPC].T)  # [128, NPC]
        in_maps.append(
            {
                "x": x,
                "sidx": _wrap16(src_all),
                "tidx": _wrap16(tgt_all),
                "deg": deg,
                "xt": xt,
                "wts": wts,
                "bstack": bstack,
                "ident": ident,
            }
        )
    return in_maps, schedule


def build_graph(schedule, total_tokens):
    nc = bacc.Bacc(dynamic_dma_scratch_size=65536)
    x_ext = nc.declare_dram_parameter("x", [N_NODES, D], F32, isOutput=False)
    sidx_ext = nc.declare_dram_parameter(
        "sidx", [128, total_tokens // 16], I16, isOutput=False
    )
    tidx_ext = nc.declare_dram_parameter(
        "tidx", [128, total_tokens // 16], I16, isOutput=False
    )
    deg_ext = nc.declare_dram_parameter("deg", [N_REL + 1, NPC], F32, isOutput=False)
    xt_ext = nc.declare_dram_parameter("xt", [D, NPC], F32, isOutput=False)
    wts_ext = nc.declare_dram_parameter("wts", [N_REL + 1, D, D], F32, isOutput=False)
    bstack_ext = nc.declare_dram_parameter("bstack", [N_REL + 1, D], F32, isOutput=False)
    ident_ext = nc.declare_dram_parameter("ident", [D, D], F32, isOutput=False)
    out_ext = nc.declare_dram_parameter("out", [D, NPC], F32, isOutput=True)
    a_out_ext = (
        nc.declare_dram_parameter("a_out", [N_REL, NPC + 1, D], F32, isOutput=True)
        if DEBUG_A
        else None
    )

    a_dram = nc.dram_tensor("a_acc", [N_REL, NPC + 1, D], F32)

    ZCOLS = 4096
    with tile.TileContext(nc) as tc:
        with (
            tc.tile_pool(name="consts", bufs=1) as consts,
            tc.tile_pool(name="stage", bufs=3) as stagep,
            tc.tile_pool(name="idx", bufs=3) as idxp,
            tc.tile_pool(name="epi", bufs=3) as epip,
            tc.tile_pool(name="psum", bufs=2, space="PSUM") as psump,
            tc.tile_pool(name="psumo", bufs=2, space="PSUM") as psumop,
        ):
            nc.gpsimd.load_library(library_config.mlp)

            zeros = consts.tile([128, ZCOLS], F32)
            nc.vector.memset(zeros[:], 0.0)
            # zero the HBM accumulator
            a_flat = a_dram[:].rearrange("r n d -> (r n) d")
            nrows = N_REL * (NPC + 1)
            ro = 0
            while ro < nrows:
                rr = min(ZCOLS, nrows - ro)
                nc.sync.dma_start(
                    out=a_flat[ro : ro + rr, :],
                    in_=zeros[:, : rr * D // 128],
                )
                ro += rr

            wt_sb = consts.tile([128, N_REL + 1, D], F32)
            nc.sync.dma_start(
                out=wt_sb[:],
                in_=wts_ext[:].rearrange("g k d -> k g d"),
            )
            bstack_sb = consts.tile([N_REL + 1, D], F32)
            nc.sync.dma_start(out=bstack_sb[:], in_=bstack_ext[:])
            ident_sb = consts.tile([D, D], F32)
            nc.sync.dma_start(out=ident_sb[:], in_=ident_ext[:])
            deg_sb = consts.tile([N_REL + 1, NPC], F32)
            nc.sync.dma_start(out=deg_sb[:], in_=deg_ext[:])

            # ---- gather / scatter-add streams ----
            for r, c, tok0, ntok in schedule:
                rows = min(SRC_CHUNK, N_NODES - c * SRC_CHUNK)
                sidx_t = idxp.tile([128, T_MAX // 16], I16, tag="sidx")
                tidx_t = idxp.tile([128, T_MAX // 16], I16, tag="tidx")
                nc.sync.dma_start(
                    out=sidx_t[:, : ntok // 16],
                    in_=sidx_ext[:, tok0 // 16 : (tok0 + ntok) // 16],
                )
                nc.sync.dma_start(
                    out=tidx_t[:, : ntok // 16],
                    in_=tidx_ext[:, tok0 // 16 : (tok0 + ntok) // 16],
                )
                stage_t = stagep.tile([128, T_MAX // 128, D], F32, tag="stage")
                nc.gpsimd.dma_gather(
                    out_ap=stage_t[:, : ntok // 128, :],
                    in_ap=x_ext[c * SRC_CHUNK : c * SRC_CHUNK + rows, :],
                    idxs_ap=sidx_t[:, : ntok // 16],
                    num_idxs=ntok,
                    num_idxs_reg=ntok,
                    elem_size=D,
                )
                nc.gpsimd.dma_scatter_add(
                    out_ap=a_dram[r],
                    in_ap=stage_t[:, : ntok // 128, :],
                    idxs_ap=tidx_t[:, : ntok // 16],
                    num_idxs=ntok,
                    num_idxs_reg=ntok,
                    elem_size=D,
                )

            # ---- epilogue ----
            n0 = 0
            while n0 < NPC:
                cw = min(NODE_CHUNK, NPC - n0)
                psum_o = psumop.tile([128, NODE_CHUNK], F32, tag="po")
                first = True
                for r in range(N_REL):
                    at_sb = epip.tile([128, NODE_CHUNK], F32, tag="at")
                    s0 = 0
                    while s0 < cw:
                        ns = min(128, cw - s0)
                        a_t = epip.tile([128, D], F32, tag="a_t")
                        nc.sync.dma_start(
                            out=a_t[:ns, :],
                            in_=a_dram[r, n0 + s0 : n0 + s0 + ns, :],
                        )
                        psum_t = psump.tile([128, 128], F32, tag="pt")
                        nc.tensor.transpose(
                            psum_t[:, :ns], a_t[:ns, :], ident_sb[:ns, :ns]
                        )
                        nc.vector.tensor_copy(
                            at_sb[:, s0 : s0 + ns], psum_t[:, :ns]
                        )
                        s0 += ns
                    nc.tensor.matmul(
                        psum_o[:, :cw],
                        wt_sb[:, r, :],
                        at_sb[:, :cw],
                        start=first,
                        stop=False,
                    )
                    first = False
                # bias terms: K=5 matmul of stacked biases against deg rows
                nc.tensor.matmul(
                    psum_o[:, :cw],
                    bstack_sb[:],
                    deg_sb[:, n0 : n0 + cw],
                    start=False,
                    stop=False,
                )
                # self-loop
                xt_t = epip.tile([128, NODE_CHUNK], F32, tag="xt")
                nc.sync.dma_start(out=xt_t[:, :cw], in_=xt_ext[:, n0 : n0 + cw])
                nc.tensor.matmul(
                    psum_o[:, :cw],
                    wt_sb[:, N_REL, :],
                    xt_t[:, :cw],
                    start=False,
                    stop=True,
                )
                out_t = epip.tile([128, NODE_CHUNK], F32, tag="out")
                nc.vector.tensor_copy(out_t[:, :cw], psum_o[:, :cw])
                nc.sync.dma_start(out=out_ext[:, n0 : n0 + cw], in_=out_t[:, :cw])
                n0 += cw
            if a_out_ext is not None:
                for r in range(N_REL):
                    ro = 0
                    while ro < NPC + 1:
                        rr = min(4096, NPC + 1 - ro)
                        nc.sync.dma_start(
                            out=a_out_ext[r, ro : ro + rr, :],
                            in_=a_dram[r, ro : ro + rr, :],
                        )
                        ro += rr
    nc.compile()
    return nc


_CACHE = {}


def kernel(x, edge_indices, W, b, W_sl, b_sl, _want_results=False, **spmd_kwargs):
    from concourse.bass_utils import run_bass_kernel_spmd

    x = np.asarray(x, dtype=np.float32)
    edge_indices = np.asarray(edge_indices).astype(np.int64)
    W = np.asarray(W, dtype=np.float32)
    b = np.asarray(b, dtype=np.float32)
    W_sl = np.asarray(W_sl, dtype=np.float32)
    b_sl = np.asarray(b_sl, dtype=np.float32)

    in_maps, schedule = prepare_inputs(x, edge_indices, W, b, W_sl, b_sl)
    total_tokens = in_maps[0]["sidx"].shape[1] * 16
    key = tuple(schedule)
    if key not in _CACHE:
        _CACHE[key] = build_graph(schedule, total_tokens)
    nc = _CACHE[key]

    res = run_bass_kernel_spmd(nc, in_maps, list(range(N_CORES)), **spmd_kwargs)
    out = np.concatenate([res.results[m]["out"].T for m in range(N_CORES)], axis=0)
    if _want_results:
        return out, res
    return out


# revision 7
# speedup vs baseline: 3.9624x; 3.9624x over previous
"""v2: scatter-free GNN message passing via membership matmuls.

Per core (nodes sharded 12500/core), per relation:
  tokens = edges sorted by (src_chunk, tgt); dma_gather (bf16, 4 SWDGE
  queues) stages x[src] as [tok%128, tok//128, feat]; per 128-token block a
  host-built dense 0/1 membership matrix M (bf16) maps tokens -> target
  columns of a zeroed 512-wide PSUM window: psum[feat, col] += X_blk.T @ M
  (all matmuls accumulate; the window is DVE-memset at open).  Windows are
  drained (DVE add) into A_r [128 feat, NPC] (bf16); per-relation epilogue
  accumulates W_r.T @ A_r into out_acc; bias/self-loop terms are a K=5
  matmul of [b_r; b_sl] against [deg_r; 1] plus W_sl.T @ x.T.

Graphs are compiled PER CORE (no SPMD padding): matmul column ranges and
drain spans are data-dependent.  Gather-descriptor latency dominates.
"""

import numpy as np
import ml_dtypes

import concourse.bass as bass
import concourse.mybir as mybir
from concourse import bacc, tile
from concourse import library_config

F32 = mybir.dt.float32
BF16 = mybir.dt.bfloat16
I16 = mybir.dt.int16
BF = ml_dtypes.bfloat16

N_NODES = 100000
D = 128
N_REL = 4
N_CORES = 8
NPC = N_NODES // N_CORES
SRC_CHUNK = 32000
N_CHUNKS = (N_NODES + SRC_CHUNK - 1) // SRC_CHUNK
T_MAX = 1024  # tokens per gather instruction (SWDGE ring limit)
WIN = 512  # psum window width (one bank)
MCAP = 1792  # max membership columns per tile (SBUF M-tile width)
NODE_CHUNK = 512
N_QUEUES = 4
SCRATCH = 65536


def plan_core(edge_indices, m):
    """Token stream + per-tile instruction plan for core m."""
    lo, hi = m * NPC, (m + 1) * NPC
    deg = np.zeros((N_REL + 1, NPC), np.float32)
    deg[N_REL] = 1.0
    src_parts, m_parts = [], []
    m_off = 0
    tok_off = 0
    rel_plans = []
    for r in range(N_REL):
        src = edge_indices[r, 0]
        tgt = edge_indices[r, 1]
        mask = (tgt >= lo) & (tgt < hi)
        s_all = src[mask].astype(np.int64)
        t_all = (tgt[mask] - lo).astype(np.int64)
        deg[r] = np.bincount(t_all, minlength=NPC).astype(np.float32)
        tiles = []
        win_id = 0
        for c in range(N_CHUNKS):
            sel = (s_all // SRC_CHUNK) == c
            sl = s_all[sel] - c * SRC_CHUNK
            tl = t_all[sel]
            order = np.argsort(tl, kind="stable")
            sl, tl = sl[order], tl[order]
            n = len(sl)
            nblk_seg = -(-max(n, 1) // 128)
            src_pad = np.zeros(nblk_seg * 128, np.int16)
            src_pad[:n] = sl.astype(np.int16)
            tgt_pad = np.full(nblk_seg * 128, -1, np.int64)
            tgt_pad[:n] = tl

            # plan blocks first (mms per block), then group into tiles
            w = None
            prev_hi = -1
            blocks = []  # per block: (mms=[(m_off,ncols,col_off,win_id,is_open)], drains=[...])
            for b in range(nblk_seg):
                tb = tgt_pad[b * 128 : b * 128 + 128]
                valid = np.nonzero(tb >= 0)[0]
                tv = tb[valid]
                mms, drains = [], []
                i = 0
                while i < len(tv):
                    t0 = int(tv[i])
                    is_open = False
                    if w is None:
                        w, prev_hi, is_open = t0, t0 - 1, True
                    elif t0 >= w + WIN:
                        drains.append((win_id, w, prev_hi - w + 1))
                        win_id += 1
                        w, prev_hi, is_open = t0, t0 - 1, True
                    j = int(np.searchsorted(tv, w + WIN))
                    p_lo, p_hi = int(tv[i]), int(tv[j - 1])
                    ncols = p_hi - p_lo + 1
                    Mm = np.zeros((128, ncols), np.float32)
                    s2 = (tb >= p_lo) & (tb <= p_hi)
                    Mm[np.nonzero(s2)[0], tb[s2] - p_lo] = 1.0
                    m_parts.append(Mm)
                    mms.append((m_off, ncols, p_lo - w, win_id, is_open))
                    m_off += ncols
                    prev_hi = max(prev_hi, p_hi)
                    i = j
                blocks.append((mms, drains))
            if w is not None:
                blocks[-1][1].append((win_id, w, prev_hi - w + 1))
                win_id += 1

            # group blocks into tiles: <= T_MAX//128 blocks and <= MCAP m-cols
            b0 = 0
            while b0 < nblk_seg:
                b1 = b0
                cols = 0
                while b1 < nblk_seg and b1 - b0 < T_MAX // 128:
                    bc = sum(mm[1] for mm in blocks[b1][0])
                    if b1 > b0 and cols + bc > MCAP:
                        break
                    cols += bc
                    b1 += 1
                tiles.append(
                    dict(
                        c=c,
                        tok0=tok_off + b0 * 128,
                        ntok=(b1 - b0) * 128,
                        blocks=[
                            (bi - b0, blocks[bi][0], blocks[bi][1])
                            for bi in range(b0, b1)
                        ],
                        mcols=cols,
                    )
                )
                b0 = b1
            src_parts.append(src_pad)
            tok_off += nblk_seg * 128
        rel_plans.append(dict(tiles=tiles, n_windows=win_id))
    src_all = np.concatenate(src_parts)
    mflat = (
        np.concatenate(m_parts, axis=1).astype(BF)
        if m_parts
        else np.zeros((128, 1), BF)
    )
    return dict(
        src=src_all,
        mflat=np.ascontiguousarray(mflat),
        deg=deg,
        rel_plans=rel_plans,
        total_tokens=tok_off,
        mcols=mflat.shape[1],
    )


def _wrap16(a):
    w = a.reshape(-1, 16).T
    return np.ascontiguousarray(np.tile(w, (8, 1)))


def build_graph_core(plan):
    total_tokens = plan["total_tokens"]
    mcols = max(plan["mcols"], 1)
    nc = bacc.Bacc(dynamic_dma_scratch_size=SCRATCH, num_swdge_queues=N_QUEUES)
    x_ext = nc.declare_dram_parameter("x", [N_NODES, D], BF16, isOutput=False)
    sidx_ext = nc.declare_dram_parameter(
        "sidx", [128, total_tokens // 16], I16, isOutput=False
    )
    m_ext = nc.declare_dram_parameter("mflat", [128, mcols], BF16, isOutput=False)
    deg_ext = nc.declare_dram_parameter("deg", [N_REL + 1, NPC], BF16, isOutput=False)
    xt_ext = nc.declare_dram_parameter("xt", [D, NPC], BF16, isOutput=False)
    wts_ext = nc.declare_dram_parameter("wts", [N_REL + 1, D, D], BF16, isOutput=False)
    bstack_ext = nc.declare_dram_parameter(
        "bstack", [N_REL + 1, D], BF16, isOutput=False
    )
    out_ext = nc.declare_dram_parameter("out", [D, NPC], BF16, isOutput=True)

    with tile.TileContext(nc) as tc:
        with (
            tc.tile_pool(name="consts", bufs=1) as consts,
            tc.tile_pool(name="stage", bufs=4) as stagep,
            tc.tile_pool(name="idx", bufs=4) as idxp,
            tc.tile_pool(name="mpool", bufs=4) as mpool,
            tc.tile_pool(name="acc", bufs=2) as accp,
            tc.tile_pool(name="win", bufs=4, space="PSUM") as winp,
            tc.tile_pool(name="pepi", bufs=2, space="PSUM") as pepip,
        ):
            nc.gpsimd.load_library(library_config.mlp)
            wt_sb = consts.tile([128, N_REL + 1, D], BF16)
            nc.sync.dma_start(
                out=wt_sb[:], in_=wts_ext[:].rearrange("g k d -> k g d")
            )
            bstack_sb = consts.tile([N_REL + 1, D], BF16)
            nc.sync.dma_start(out=bstack_sb[:], in_=bstack_ext[:])
            out_acc = consts.tile([D, NPC], BF16)

            # bias + self-loop (deg/xt streamed per chunk)
            n0 = 0
            while n0 < NPC:
                cw = min(NODE_CHUNK, NPC - n0)
                deg_t = idxp.tile([N_REL + 1, NODE_CHUNK], BF16, tag="deg")
                nc.sync.dma_start(out=deg_t[:, :cw], in_=deg_ext[:, n0 : n0 + cw])
                xt_t = stagep.tile([D, NODE_CHUNK], BF16, tag="xt")
                nc.sync.dma_start(out=xt_t[:, :cw], in_=xt_ext[:, n0 : n0 + cw])
                pe = pepip.tile([128, NODE_CHUNK], F32, tag="pe")
                nc.tensor.matmul(
                    pe[:, :cw], bstack_sb[:], deg_t[:, :cw],
                    start=True, stop=False,
                )
                nc.tensor.matmul(
                    pe[:, :cw], wt_sb[:, N_REL, :], xt_t[:, :cw],
                    start=False, stop=True,
                )
                nc.any.tensor_copy(out_acc[:, n0 : n0 + cw], pe[:, :cw])
                n0 += cw

            gq = 0
            for r in range(N_REL):
                rp = plan["rel_plans"][r]
                a_sb = accp.tile([128, NPC], BF16, tag="a")
                nc.vector.memset(a_sb[:], 0.0)
                win_tiles = {}
                for t in rp["tiles"]:
                    ntok = t["ntok"]
                    c = t["c"]
                    rows = min(SRC_CHUNK, N_NODES - c * SRC_CHUNK)
                    sidx_t = idxp.tile([128, T_MAX // 16], I16, tag="sidx")
                    nc.sync.dma_start(
                        out=sidx_t[:, : ntok // 16],
                        in_=sidx_ext[:, t["tok0"] // 16 : (t["tok0"] + ntok) // 16],
                    )
                    stage_t = stagep.tile([128, T_MAX // 128, D], BF16, tag="st")
                    nc.gpsimd.dma_gather(
                        out_ap=stage_t[:, : ntok // 128, :],
                        in_ap=x_ext[c * SRC_CHUNK : c * SRC_CHUNK + rows, :],
                        idxs_ap=sidx_t[:, : ntok // 16],
                        num_idxs=ntok,
                        num_idxs_reg=ntok,
                        elem_size=D,
                        queue_num=gq % N_QUEUES,
                        single_packet=False,
                    )
                    gq += 1
                    mo0 = None
                    for _, bmms, _ in t["blocks"]:
                        if bmms:
                            mo0 = bmms[0][0]
                            break
                    if mo0 is not None:
                        mt = mpool.tile([128, MCAP], BF16, tag="m")
                        nc.sync.dma_start(
                            out=mt[:, : t["mcols"]],
                            in_=m_ext[:, mo0 : mo0 + t["mcols"]],
                        )
                    for blk, bmms, bdrains in t["blocks"]:
                        for mo, ncols, col_off, wid, is_open in bmms:
                            if is_open:
                                win_tiles[wid] = winp.tile([128, WIN], F32, tag="w", name=f"win{wid%16}")
                                nc.vector.memset(win_tiles[wid][:], 0.0)
                            nc.tensor.matmul(
                                win_tiles[wid][:, col_off : col_off + ncols],
                                stage_t[:, blk, :],
                                mt[:, mo - mo0 : mo - mo0 + ncols],
                                start=False,
                                stop=False,
                                skip_group_check=True,
                            )
                        for wid, wbase, length in bdrains:
                            wt_ = win_tiles.pop(wid)
                            nc.any.tensor_tensor(
                                out=a_sb[:, wbase : wbase + length],
                                in0=a_sb[:, wbase : wbase + length],
                                in1=wt_[:, :length],
                                op=mybir.AluOpType.add,
                            )
                # epilogue for relation r
                n0 = 0
                while n0 < NPC:
                    cw = min(NODE_CHUNK, NPC - n0)
                    pe = pepip.tile([128, NODE_CHUNK], F32, tag="pe")
                    nc.tensor.matmul(
                        pe[:, :cw], wt_sb[:, r, :], a_sb[:, n0 : n0 + cw],
                        start=True, stop=True,
                    )
                    nc.any.tensor_tensor(
                        out=out_acc[:, n0 : n0 + cw],
                        in0=out_acc[:, n0 : n0 + cw],
                        in1=pe[:, :cw],
                        op=mybir.AluOpType.add,
                    )
                    n0 += cw
            n0 = 0
            while n0 < NPC:
                cw = min(4096, NPC - n0)
                nc.sync.dma_start(
                    out=out_ext[:, n0 : n0 + cw], in_=out_acc[:, n0 : n0 + cw]
                )
                n0 += cw
    nc.compile()
    return nc


def prepare_all(x, edge_indices, W, b, W_sl, b_sl):
    x = np.ascontiguousarray(x, dtype=np.float32)
    x_bf = x.astype(BF)
    wts = np.ascontiguousarray(
        np.stack([W[r].T for r in range(N_REL)] + [W_sl.T])
    ).astype(BF)
    bstack = np.ascontiguousarray(
        np.concatenate([b, b_sl[None, :]], axis=0)
    ).astype(BF)
    plans, in_maps = [], []
    for m in range(N_CORES):
        p = plan_core(edge_indices, m)
        plans.append(p)
        xt = np.ascontiguousarray(x[m * NPC : (m + 1) * NPC].T).astype(BF)
        in_maps.append(
            {
                "x": x_bf,
                "sidx": _wrap16(p["src"]),
                "mflat": p["mflat"],
                "deg": p["deg"].astype(BF),
                "xt": xt,
                "wts": wts,
                "bstack": bstack,
            }
        )
    return plans, in_maps


def make_runner(nc, in_map, device):
    """jit-compiled single-core executable pinned to `device`."""
    import jax
    import concourse.mybir as mb
    from concourse import bass2jax
    from concourse.bass2jax import _bass_exec_p, partition_id_tensor

    bass2jax.install_neuronx_cc_hook()
    partition_name = nc.partition_id_tensor.name if nc.partition_id_tensor else None
    in_names, out_names, out_avals = [], [], []
    for alloc in nc.m.functions[0].allocations:
        if not isinstance(alloc, mb.MemoryLocationSet):
            continue
        name = alloc.memorylocations[0].name
        if alloc.kind == "ExternalInput":
            if name != partition_name:
                in_names.append(name)
        elif alloc.kind == "ExternalOutput":
            out_names.append(name)
            out_avals.append(
                jax.core.ShapedArray(tuple(alloc.tensor_shape), mb.dt.np(alloc.dtype))
            )
    all_in = list(in_names) + list(out_names)
    if partition_name is not None:
        all_in.append(partition_name)

    def _body(*a):
        ops = list(a)
        if partition_name is not None:
            ops.append(partition_id_tensor())
        return tuple(
            _bass_exec_p.bind(
                *ops,
                out_avals=tuple(out_avals),
                in_names=tuple(all_in),
                out_names=tuple(out_names),
                lowering_input_output_aliases=(),
                sim_require_finite=True,
                sim_require_nnan=True,
                nc=nc,
            )
        )

    import jax

    arrs = [jax.device_put(np.asarray(in_map[k]), device) for k in in_names]
    arrs += [
        jax.device_put(np.zeros(av.shape, av.dtype), device) for av in out_avals
    ]
    return jax.jit(_body, keep_unused=True), arrs


def kernel(x, edge_indices, W, b, W_sl, b_sl, _ret_all=False):
    import jax

    x = np.asarray(x, dtype=np.float32)
    edge_indices = np.asarray(edge_indices).astype(np.int64)
    plans, in_maps = prepare_all(
        x, edge_indices, np.asarray(W, np.float32), np.asarray(b, np.float32),
        np.asarray(W_sl, np.float32), np.asarray(b_sl, np.float32),
    )
    ncs = [build_graph_core(p) for p in plans]
    devices = jax.devices()[:N_CORES]
    runners = [
        make_runner(ncs[m], in_maps[m], devices[m]) for m in range(N_CORES)
    ]
    outs = [fn(*arrs) for fn, arrs in runners]
    jax.block_until_ready(outs)
    out = np.concatenate([np.asarray(o[0]).T for o in outs], axis=0).astype(
        np.float32
    )
    if _ret_all:
        return out, (ncs, runners, plans, in_maps)
    return out
